# revision 16
# baseline (speedup 1.0000x reference)
"""AlignmentModel Trainium2 kernel: 8-core data-parallel over the B=1024 pair axis.

Per core: 128 pairs. Host gathers token embeddings (pure indexing) into a
transposed [D, sent*tok] layout; device does encoder MLP, cosine-cost Gram,
non-log-domain Sinkhorn (pairs on the partition axis), attend, compare MLP,
masked aggregation, and the output MLP. Outputs concatenated on host.
"""

import numpy as np
from contextlib import ExitStack

N_CORES = 8
NPAIR = 128          # pairs per core
L = 128              # tokens per sentence
D = 512              # embedding dim
H = 256              # hidden dim
NS = 2 * NPAIR       # sentences per core (rows then cols)
NTOK = NS * L        # 32768 token slots per core
ITERS = 25           # sinkhorn iterations (validated vs 50-iter reference)
EPS_SCALE = 20.0     # 1/EPS with EPS=0.05
NEGM = -1000.0       # additive mask; exp(-1020) == 0 in f32

_CACHE = {}


def _build_graph():
    import concourse.bass as bass
    import concourse.tile as tile
    from concourse import bacc, mybir

    f32 = mybir.dt.float32
    bf16 = mybir.dt.bfloat16
    X = mybir.AxisListType.X
    MULT = mybir.AluOpType.add  # placeholder, replaced below
    MULT = mybir.AluOpType.mult
    ADD = mybir.AluOpType.add
    AF = mybir.ActivationFunctionType

    nc = bacc.Bacc(None, target_bir_lowering=False, debug=False)

    # ---------------- parameters ----------------
    xT = nc.declare_dram_parameter("xT", [D, NTOK], f32, isOutput=False)
    a_p = nc.declare_dram_parameter("a_mat", [NPAIR, L], f32, isOutput=False)
    b_p = nc.declare_dram_parameter("b_mat", [NPAIR, L], f32, isOutput=False)
    cm05_p = nc.declare_dram_parameter("cm05", [NPAIR, L], f32, isOutput=False)
    negm_p = nc.declare_dram_parameter("negm", [NPAIR, L], f32, isOutput=False)
    rmf_p = nc.declare_dram_parameter("rm_flat", [1, NPAIR * L], f32, isOutput=False)
    cmf_p = nc.declare_dram_parameter("cm_flat", [1, NPAIR * L], f32, isOutput=False)
    wa1_p = nc.declare_dram_parameter("w_a1", [D, H], f32, isOutput=False)
    wa2_p = nc.declare_dram_parameter("w_a2", [H, H], f32, isOutput=False)
    wc1_p = nc.declare_dram_parameter("w_c1", [2 * H, H], f32, isOutput=False)
    wc2_p = nc.declare_dram_parameter("w_c2", [H, H], f32, isOutput=False)
    wg1_p = nc.declare_dram_parameter("w_g1", [2 * H, H], f32, isOutput=False)
    wg2_p = nc.declare_dram_parameter("w_g2", [H, H], f32, isOutput=False)
    wo_p = nc.declare_dram_parameter("w_o", [H, 3], f32, isOutput=False)
    ba1_p = nc.declare_dram_parameter("b_a1", [H, 1], f32, isOutput=False)
    ba2_p = nc.declare_dram_parameter("b_a2", [H, 1], f32, isOutput=False)
    bc1_p = nc.declare_dram_parameter("b_c1", [H, 1], f32, isOutput=False)
    bc2_p = nc.declare_dram_parameter("b_c2", [H, 1], f32, isOutput=False)
    bg1_p = nc.declare_dram_parameter("b_g1", [H, 1], f32, isOutput=False)
    bg2_p = nc.declare_dram_parameter("b_g2", [H, 1], f32, isOutput=False)
    bo_p = nc.declare_dram_parameter("b_o", [3, 1], f32, isOutput=False)
    out_p = nc.declare_dram_parameter("out_p", [NPAIR, L, L], f32, isOutput=True)
    out_lg = nc.declare_dram_parameter("out_lg", [3, NPAIR], f32, isOutput=True)

    # ---------------- dram scratch ----------------
    encT_f = nc.dram_tensor("encT_f", [2, 128, NTOK], f32)        # [h_tile][h][s*L+t]
    encT_b = nc.dram_tensor("encT_b", [2, 128, NTOK], bf16)
    encNT_b = nc.dram_tensor("encNT_b", [NS, 128, H], bf16)       # [sent][tok][h]
    t1A_d = nc.dram_tensor("t1A_d", [NPAIR, L, L], f32)           # scaled gram, pair-major
    prc_d = nc.dram_tensor("prc_d", [NPAIR, L, L], bf16)          # P  [p][r][c]
    pcr_d = nc.dram_tensor("pcr_d", [NPAIR, L, L], bf16)          # P^T [p][c][r]
    invsc_d = nc.dram_tensor("invsc_d", [NTOK // 512, 512], f32)  # per-chunk 20/norm, (s,t)
    invt_d = nc.dram_tensor("invt_d", [128, 128], f32)            # col-sent invn, [tok, s]

    NCH = NTOK // 512  # 64 chunks of 512 tokens (4 sentences)

    with tile.TileContext(nc) as tc, ExitStack() as top:
        wpool = top.enter_context(tc.tile_pool(name="weights", bufs=1))
        persist = top.enter_context(tc.tile_pool(name="persist", bufs=1))

        # --- load weights (f32 + bf16 copies) and biases ---
        def load_w(param, kt, name, want_bf):
            ws, wbs = [], []
            for k in range(kt):
                w = wpool.tile([128, param.shape[1]], f32, tag=f"{name}{k}", name=f"{name}{k}")
                nc.sync.dma_start(w[:], param[k * 128:(k + 1) * 128, :])
                ws.append(w)
                if want_bf:
                    wb = wpool.tile([128, param.shape[1]], bf16, tag=f"{name}b{k}", name=f"{name}b{k}")
                    nc.vector.tensor_copy(wb[:], w[:])
                    wbs.append(wb)
            return ws, wbs

        wa1, _ = load_w(wa1_p, 4, "wa1", False)
        wa2, _ = load_w(wa2_p, 2, "wa2", False)
        _, wc1b = load_w(wc1_p, 4, "wc1", True)
        _, wc2b = load_w(wc2_p, 2, "wc2", True)
        _, wg1b = load_w(wg1_p, 4, "wg1", True)
        _, wg2b = load_w(wg2_p, 2, "wg2", True)
        _, wob = load_w(wo_p, 2, "wo", True)

        def load_b(param, kt, name):
            bs = []
            for k in range(kt):
                b = wpool.tile([128, 1], f32, tag=f"{name}{k}", name=f"{name}{k}")
                nc.sync.dma_start(b[:], param[k * 128:(k + 1) * 128, :])
                bs.append(b)
            return bs

        ba1 = load_b(ba1_p, 2, "ba1")
        ba2 = load_b(ba2_p, 2, "ba2")
        bc1 = load_b(bc1_p, 2, "bc1")
        bc2 = load_b(bc2_p, 2, "bc2")
        bg1 = load_b(bg1_p, 2, "bg1")
        bg2 = load_b(bg2_p, 2, "bg2")
        bo_t = wpool.tile([3, 1], f32, tag="bo", name="bo")
        nc.sync.dma_start(bo_t[:], bo_p[:, :])

        ones_t = wpool.tile([128, 1], f32, tag="ones", name="ones")
        nc.vector.memset(ones_t[:], 1.0)

        invn20 = persist.tile([128, NS], f32, tag="invn20", name="invn20")  # [tok, sent] 20/(||enc||+1e-8)

        # ================= Phase 1: encoder =================
        with ExitStack() as ph, tc.tile_pool(name="p1", bufs=3) as p1, \
                tc.tile_pool(name="p1psum", bufs=3, space="PSUM") as pp1:
            for ch in range(NCH):
                c0 = ch * 512
                xt = [p1.tile([128, 512], f32, tag=f"xt{k}", name=f"xt{k}") for k in range(4)]
                for k in range(4):
                    nc.sync.dma_start(xt[k][:], xT[k * 128:(k + 1) * 128, c0:c0 + 512])
                enc1 = []
                for m in range(2):
                    ps = pp1.tile([128, 512], f32, tag="ps_a", name="ps_a")
                    for k in range(4):
                        nc.tensor.matmul(ps[:], wa1[k][:, m * 128:(m + 1) * 128],
                                         xt[k][:], start=(k == 0), stop=(k == 3))
                    e1 = p1.tile([128, 512], f32, tag=f"enc1_{m}", name=f"enc1_{m}")
                    nc.scalar.activation(e1[:], ps[:], AF.Relu, bias=ba1[m][:])
                    enc1.append(e1)
                enc2 = []
                enc2b = []
                for m in range(2):
                    ps = pp1.tile([128, 512], f32, tag="ps_a", name="ps_a")
                    for k in range(2):
                        nc.tensor.matmul(ps[:], wa2[k][:, m * 128:(m + 1) * 128],
                                         enc1[k][:], start=(k == 0), stop=(k == 1))
                    e2 = p1.tile([128, 512], f32, tag=f"enc2_{m}", name=f"enc2_{m}")
                    nc.scalar.activation(e2[:], ps[:], AF.Relu, bias=ba2[m][:])
                    e2b = p1.tile([128, 512], bf16, tag=f"enc2b_{m}", name=f"enc2b_{m}")
                    nc.vector.tensor_copy(e2b[:], e2[:])
                    nc.sync.dma_start(encT_f[m, :, c0:c0 + 512], e2[:])
                    nc.sync.dma_start(encT_b[m, :, c0:c0 + 512], e2b[:])
                    enc2.append(e2)
                    enc2b.append(e2b)
                # NT transposes (bf16, xbar) + spill
                for s in range(4):
                    ntile = p1.tile([128, H], bf16, tag=f"nt{s}", name=f"nt{s}")
                    for m in range(2):
                        nc.sync.dma_start_transpose(
                            ntile[:, m * 128:(m + 1) * 128],
                            enc2b[m][:, s * 128:(s + 1) * 128])
                    nc.sync.dma_start(encNT_b[ch * 4 + s], ntile[:])
                # norms (f32): sum_h enc2^2 via ones-matmul
                psn = pp1.tile([1, 512], f32, tag="ps_n", name="ps_n")
                for m in range(2):
                    sq = p1.tile([128, 512], f32, tag="sq", name="sq")
                    nc.scalar.activation(sq[:], enc2[m][:], AF.Square)
                    nc.tensor.matmul(psn[:], ones_t[:], sq[:],
                                     start=(m == 0), stop=(m == 1))
                nrm = p1.tile([1, 512], f32, tag="nrm", name="nrm")
                nc.scalar.activation(nrm[:], psn[:], AF.Sqrt)
                nc.vector.tensor_scalar_add(nrm[:], nrm[:], 1e-8)
                inv = p1.tile([1, 512], f32, tag="inv", name="inv")
                nc.vector.reciprocal(inv[:], nrm[:])
                nc.vector.tensor_scalar_mul(inv[:], inv[:], EPS_SCALE)
                nc.sync.dma_start(invsc_d[ch:ch + 1, :], inv[:])
            # gather chunk rows -> invn20 [tok, sent] (DRAM source: strides free)
            nc.sync.dma_start(
                invn20[:], invsc_d[:].rearrange("ch (s t) -> t (ch s)", s=4))

        # invnc_cm = transpose(invn20[:, cols]) * cm05   (cm05 = cmask*0.05 folds /20)
        invnc_cm = persist.tile([NPAIR, L], f32, tag="invnc_cm", name="invnc_cm")
        with tc.tile_pool(name="ptr", bufs=1) as ptr:
            cm05 = ptr.tile([NPAIR, L], f32, tag="cm05", name="cm05")
            nc.sync.dma_start(cm05[:], cm05_p[:, :])
            nc.sync.dma_start(invt_d[:], invn20[:, NPAIR:NS])
            nc.sync.dma_start(invnc_cm[:], invt_d[:].rearrange("t s -> s t"))
            nc.vector.tensor_tensor(invnc_cm[:], invnc_cm[:], cm05[:], MULT)

        # ================= Phase 2: Gram + K build =================
        with ExitStack() as ph, tc.tile_pool(name="p2", bufs=4) as p2, \
                tc.tile_pool(name="p2psum", bufs=4, space="PSUM") as pp2:
            for p in range(NPAIR):
                rt = [p2.tile([128, 128], f32, tag=f"rt{m}", name=f"rt{m}") for m in range(2)]
                ct = [p2.tile([128, 128], f32, tag=f"ct{m}", name=f"ct{m}") for m in range(2)]
                for m in range(2):
                    nc.sync.dma_start(rt[m][:], encT_f[m, :, p * L:(p + 1) * L])
                    nc.sync.dma_start(ct[m][:],
                                      encT_f[m, :, (NPAIR + p) * L:(NPAIR + p + 1) * L])
                ps = pp2.tile([128, 128], f32, tag="ps_g", name="ps_g")
                for m in range(2):
                    nc.tensor.matmul(ps[:], rt[m][:], ct[m][:],
                                     start=(m == 0), stop=(m == 1))
                t1 = p2.tile([128, 128], f32, tag="t1", name="t1")
                nc.scalar.activation(t1[:], ps[:], AF.Copy, scale=invn20[:, p:p + 1])
                nc.sync.dma_start(t1A_d[p], t1[:])

        # ================= Phase 3: K + sinkhorn (pairs on partitions) ========
        with ExitStack() as ph:
            spool = ph.enter_context(tc.tile_pool(name="spool", bufs=1))
            kp_e = ph.enter_context(tc.tile_pool(name="kp_e", bufs=1))
            kp_k_cm = tc.tile_pool(name="kp_k", bufs=1)
            kp_k = kp_k_cm.__enter__()
            K_A = kp_k.tile([NPAIR, L * L], bf16, tag="K_A", name="K_A")
            E = kp_e.tile([NPAIR, L * L], bf16, tag="E", name="E")
            with tc.tile_pool(name="kp_t1", bufs=1) as kp_t1:
                t1A = kp_t1.tile([NPAIR, L * L], f32, tag="t1A", name="t1A")
                nc.sync.dma_start(t1A[:], t1A_d[:].rearrange("p r c -> p (r c)"))
                t1A3 = t1A[:].rearrange("p (r c) -> p r c", r=L)
                nc.vector.tensor_tensor(
                    t1A3, t1A3, invnc_cm[:, None, :].broadcast_to([NPAIR, L, L]), MULT)
                negm = spool.tile([NPAIR, L], f32, tag="negm", name="negm")
                nc.sync.dma_start(negm[:], negm_p[:, :])
                nc.vector.tensor_tensor(
                    t1A3, t1A3, negm[:, None, :].broadcast_to([NPAIR, L, L]), ADD)
                neg20 = spool.tile([NPAIR, 1], f32, tag="neg20", name="neg20")
                nc.vector.memset(neg20[:], -EPS_SCALE)
                nc.scalar.activation(K_A[:], t1A[:], AF.Exp, bias=neg20[:])
            K3 = K_A[:].rearrange("p (r c) -> p r c", r=L)
            E3 = E[:].rearrange("p (r c) -> p r c", r=L)
            E3s = E[:].rearrange("p (r c) -> p c r", r=L)

            a_t = spool.tile([NPAIR, L], f32, tag="a_t", name="a_t")
            b_t = spool.tile([NPAIR, L], f32, tag="b_t", name="b_t")
            nc.sync.dma_start(a_t[:], a_p[:, :])
            nc.sync.dma_start(b_t[:], b_p[:, :])
            S = spool.tile([NPAIR, L], f32, tag="S", name="S")
            Sr = spool.tile([NPAIR, L], f32, tag="Sr", name="Sr")
            eu = spool.tile([NPAIR, L], f32, tag="eu", name="eu")
            ev = spool.tile([NPAIR, L], f32, tag="ev", name="ev")
            eub = spool.tile([NPAIR, L], bf16, tag="eub", name="eub")
            evb = spool.tile([NPAIR, L], bf16, tag="evb", name="evb")
            nc.vector.memset(evb[:], 1.0)

            for it in range(ITERS):
                nc.vector.tensor_tensor(
                    E3, K3, evb[:, None, :].broadcast_to([NPAIR, L, L]), MULT)
                nc.vector.tensor_reduce(S[:], E3, X, ADD)
                nc.vector.tensor_scalar_max(S[:], S[:], 1e-30)
                nc.vector.reciprocal(Sr[:], S[:])
                nc.vector.tensor_tensor(eu[:], a_t[:], Sr[:], MULT)
                nc.vector.tensor_copy(eub[:], eu[:])
                nc.vector.tensor_tensor(
                    E3, K3, eub[:, :, None].broadcast_to([NPAIR, L, L]), MULT)
                nc.vector.tensor_reduce(S[:], E3s, X, ADD)
                nc.vector.tensor_scalar_max(S[:], S[:], 1e-30)
                nc.vector.reciprocal(Sr[:], S[:])
                nc.vector.tensor_tensor(ev[:], b_t[:], Sr[:], MULT)
                nc.vector.tensor_copy(evb[:], ev[:])

            # finals: P = K*eu*ev in three layouts (Pa lands in E, K then dead)
            nc.vector.tensor_tensor(
                E3, K3, eub[:, :, None].broadcast_to([NPAIR, L, L]), MULT)
            kp_k_cm.__exit__(None, None, None)  # K_A dead; free its 4.2MB
            with tc.tile_pool(name="kp_pa", bufs=1) as kp_pa:
                PA = kp_pa.tile([NPAIR, L * L], f32, tag="PA", name="PA")
                PA3 = PA[:].rearrange("p (r c) -> p r c", r=L)
                nc.vector.tensor_tensor(
                    PA3, E3, evb[:, None, :].broadcast_to([NPAIR, L, L]), MULT)
                nc.sync.dma_start(out_p[:], PA[:].rearrange("p (r c) -> p r c", r=L))
            with tc.tile_pool(name="kp_prc", bufs=1) as kp_prc:
                Prc = kp_prc.tile([NPAIR, L * L], bf16, tag="Prc", name="Prc")
                nc.vector.tensor_tensor(
                    Prc[:].rearrange("p (r c) -> p r c", r=L), E3,
                    evb[:, None, :].broadcast_to([NPAIR, L, L]), MULT)
                nc.sync.dma_start(prc_d[:], Prc[:].rearrange("p (r c) -> p r c", r=L))
                Pcr = kp_prc.tile([NPAIR, L * L], bf16, tag="Pcr", name="Pcr")
                nc.vector.tensor_tensor(
                    Pcr[:].rearrange("p (c r) -> p r c", c=L), E3,
                    evb[:, None, :].broadcast_to([NPAIR, L, L]), MULT)
                nc.sync.dma_start(pcr_d[:], Pcr[:].rearrange("p (c r) -> p c r", c=L))

        # ================= Phase 4: attend + compare + agg =================
        aggb = []
        with ExitStack() as ph:
            rpool = ph.enter_context(tc.tile_pool(name="reps", bufs=1))
            repr_b = rpool.tile([128, NPAIR * L], bf16, tag="repr", name="repr")
            repc_b = rpool.tile([128, NPAIR * L], bf16, tag="repc", name="repc")
            with tc.tile_pool(name="reptmp", bufs=1) as rtmp:
                repf = rtmp.tile([128, NPAIR * L], f32, tag="repf", name="repf")
                nc.sync.dma_start(repf[:], rmf_p[0:1, :].to_broadcast([128, NPAIR * L]))
                nc.vector.tensor_copy(repr_b[:], repf[:])
                repf2 = rtmp.tile([128, NPAIR * L], f32, tag="repf", name="repf")
                nc.sync.dma_start(repf2[:], cmf_p[0:1, :].to_broadcast([128, NPAIR * L]))
                nc.vector.tensor_copy(repc_b[:], repf2[:])

            aggp = ph.enter_context(tc.tile_pool(name="agg", bufs=1))
            aggR = [aggp.tile([128, NPAIR], f32, tag=f"aggR{m}", name=f"aggR{m}") for m in range(2)]
            aggC = [aggp.tile([128, NPAIR], f32, tag=f"aggC{m}", name=f"aggC{m}") for m in range(2)]

            p4 = ph.enter_context(tc.tile_pool(name="p4", bufs=3))
            pp4 = ph.enter_context(tc.tile_pool(name="p4psum", bufs=2, space="PSUM"))
            for chb in range(NPAIR // 4):
                rmov = [p4.tile([128, 512], bf16, tag=f"rmov{k}", name=f"rmov{k}") for k in range(4)]
                cmov = [p4.tile([128, 512], bf16, tag=f"cmov{k}", name=f"cmov{k}") for k in range(4)]
                for i in range(4):
                    p = chb * 4 + i
                    prc_t = p4.tile([128, 128], bf16, tag="prc_t", name="prc_t")
                    pcr_t = p4.tile([128, 128], bf16, tag="pcr_t", name="pcr_t")
                    nc.sync.dma_start(prc_t[:], prc_d[p])
                    nc.sync.dma_start(pcr_t[:], pcr_d[p])
                    rnt = p4.tile([128, H], bf16, tag="rnt", name="rnt")
                    cnt = p4.tile([128, H], bf16, tag="cnt", name="cnt")
                    nc.sync.dma_start(rnt[:], encNT_b[p])
                    nc.sync.dma_start(cnt[:], encNT_b[NPAIR + p])
                    # moving k-tiles 0,1 = encT rows/cols slices
                    for m in range(2):
                        nc.sync.dma_start(rmov[m][:, i * 128:(i + 1) * 128],
                                          encT_b[m, :, p * L:(p + 1) * L])
                        nc.sync.dma_start(cmov[m][:, i * 128:(i + 1) * 128],
                                          encT_b[m, :, (NPAIR + p) * L:(NPAIR + p + 1) * L])
                    # attend: att_colT = colsNT^T @ Pcr ; att_rowT = rowsNT^T @ Prc
                    for m in range(2):
                        ps = pp4.tile([128, 128], f32, tag="ps_at", name="ps_at")
                        nc.tensor.matmul(ps[:], cnt[:, m * 128:(m + 1) * 128], pcr_t[:])
                        nc.scalar.activation(rmov[2 + m][:, i * 128:(i + 1) * 128],
                                             ps[:], AF.Copy)
                        ps2 = pp4.tile([128, 128], f32, tag="ps_at", name="ps_at")
                        nc.tensor.matmul(ps2[:], rnt[:, m * 128:(m + 1) * 128], prc_t[:])
                        nc.scalar.activation(cmov[2 + m][:, i * 128:(i + 1) * 128],
                                             ps2[:], AF.Copy)
                # compare both sides
                for side, (mov, rep, agg) in enumerate(
                        [(rmov, repr_b, aggR), (cmov, repc_b, aggC)]):
                    c1 = []
                    for m in range(2):
                        ps = pp4.tile([128, 512], f32, tag="ps_c", name="ps_c")
                        for k in range(4):
                            nc.tensor.matmul(ps[:], wc1b[k][:, m * 128:(m + 1) * 128],
                                             mov[k][:], start=(k == 0), stop=(k == 3))
                        t = p4.tile([128, 512], bf16, tag=f"c1_{m}", name=f"c1_{m}")
                        nc.scalar.activation(t[:], ps[:], AF.Relu, bias=bc1[m][:])
                        c1.append(t)
                    for m in range(2):
                        ps = pp4.tile([128, 512], f32, tag="ps_c", name="ps_c")
                        for k in range(2):
                            nc.tensor.matmul(ps[:], wc2b[k][:, m * 128:(m + 1) * 128],
                                             c1[k][:], start=(k == 0), stop=(k == 1))
                        cr = p4.tile([128, 512], bf16, tag=f"cr_{m}", name=f"cr_{m}")
                        nc.scalar.activation(cr[:], ps[:], AF.Relu, bias=bc2[m][:])
                        nc.vector.tensor_tensor(
                            cr[:], cr[:], rep[:, chb * 512:(chb + 1) * 512], MULT)
                        nc.vector.tensor_reduce(
                            agg[m][:, chb * 4:chb * 4 + 4],
                            cr[:].rearrange("h (s t) -> h s t", s=4), X, ADD)

            for k, agg in enumerate(aggR + aggC):
                ab = aggp.tile([128, NPAIR], bf16, tag=f"aggb{k}", name=f"aggb{k}")
                nc.vector.tensor_copy(ab[:], agg[:])
                aggb.append(ab)

            # output MLP
            g1 = []
            for m in range(2):
                ps = pp4.tile([128, NPAIR], f32, tag="ps_g1", name="ps_g1")
                for k in range(4):
                    nc.tensor.matmul(ps[:], wg1b[k][:, m * 128:(m + 1) * 128],
                                     aggb[k][:], start=(k == 0), stop=(k == 3))
                t = p4.tile([128, NPAIR], bf16, tag=f"g1_{m}", name=f"g1_{m}")
                nc.scalar.activation(t[:], ps[:], AF.Relu, bias=bg1[m][:])
                g1.append(t)
            g2 = []
            for m in range(2):
                ps = pp4.tile([128, NPAIR], f32, tag="ps_g1", name="ps_g1")
                for k in range(2):
                    nc.tensor.matmul(ps[:], wg2b[k][:, m * 128:(m + 1) * 128],
                                     g1[k][:], start=(k == 0), stop=(k == 1))
                t = p4.tile([128, NPAIR], bf16, tag=f"g2_{m}", name=f"g2_{m}")
                nc.scalar.activation(t[:], ps[:], AF.Relu, bias=bg2[m][:])
                g2.append(t)
            ps3 = pp4.tile([3, NPAIR], f32, tag="ps_o", name="ps_o")
            for k in range(2):
                nc.tensor.matmul(ps3[:], wob[k][:], g2[k][:],
                                 start=(k == 0), stop=(k == 1))
            lgt = p4.tile([3, NPAIR], f32, tag="lgt", name="lgt")
            nc.scalar.add(lgt[:], ps3[:], bo_t[:])
            nc.sync.dma_start(out_lg[:], lgt[:])

    nc.compile()
    return nc


def _host_prep(data, row_idx, col_idx, emb, weights):
    """Build per-core input maps. Pure indexing/layout work on host."""
    data = np.asarray(data)
    mask = (data != 0).astype(np.float32)                 # [N, L]
    in_maps = []
    for c in range(N_CORES):
        ridx = np.asarray(row_idx[c * NPAIR:(c + 1) * NPAIR])
        cidx = np.asarray(col_idx[c * NPAIR:(c + 1) * NPAIR])
        sents = np.concatenate([ridx, cidx])              # [256]
        toks = data[sents]                                # [256, L]
        xg = emb[toks.reshape(-1)]                        # [NTOK, D]
        xT = np.ascontiguousarray(xg.T.astype(np.float32))  # [D, NTOK]
        rmask = mask[ridx]                                # [128, L]
        cmask = mask[cidx]
        n = rmask.sum(-1, keepdims=True)
        m = cmask.sum(-1, keepdims=True)
        im = dict(
            xT=xT,
            a_mat=np.ascontiguousarray(rmask / n),
            b_mat=np.ascontiguousarray(cmask / m),
            cm05=np.ascontiguousarray(cmask * (1.0 / EPS_SCALE)),
            negm=np.ascontiguousarray((1.0 - cmask) * NEGM),
            rm_flat=np.ascontiguousarray(rmask.reshape(1, -1)),
            cm_flat=np.ascontiguousarray(cmask.reshape(1, -1)),
        )
        im.update(weights)
        in_maps.append(im)
    return in_maps


def _install_profile_shim():
    """Recreate the missing antenv.axon_hooks module + NTFF ctypes hook."""
    import sys, types, ctypes, contextlib
    if "antenv.axon_hooks" in sys.modules:
        return
    so_path = "/opt/axon/libaxon_pjrt.so"
    mod = types.ModuleType("antenv.axon_hooks")
    _state = {"hook": None}
    mod.set_axon_ntff_profile_hook = lambda h: _state.__setitem__("hook", h)
    mod.get_axon_ntff_profile_hook = lambda: _state["hook"]
    try:
        lib = ctypes.CDLL(so_path)
        if hasattr(lib, "axon_start_nrt_profile"):
            lib.axon_start_nrt_profile.argtypes = [
                ctypes.POINTER(ctypes.c_int64), ctypes.c_size_t]
            lib.axon_start_nrt_profile.restype = ctypes.c_int64
            lib.axon_stop_nrt_profile.argtypes = [ctypes.c_char_p]
            lib.axon_stop_nrt_profile.restype = ctypes.c_int64

            @contextlib.contextmanager
            def _hook(output_dir, device_ids):
                import jax
                jax.devices()
                if device_ids:
                    ids = (ctypes.c_int64 * len(device_ids))(*device_ids)
                    rc = lib.axon_start_nrt_profile(ids, len(device_ids))
                else:
                    rc = lib.axon_start_nrt_profile(None, 0)
                if rc != 0:
                    raise RuntimeError(f"axon_start_nrt_profile rc={rc}")
                try:
                    yield
                finally:
                    n = lib.axon_stop_nrt_profile(str(output_dir).encode())
                    print(f"profile: {n} file(s) written to {output_dir}")

            _state["hook"] = _hook
    except OSError:
        pass
    sys.modules["antenv.axon_hooks"] = mod


def kernel(data, row_idx, col_idx, emb, W_a1, b_a1, W_a2, b_a2,
           W_c1, b_c1, W_c2, b_c2, W_g1, b_g1, W_g2, b_g2, W_o, b_o,
           _trace=False, _trace_kwargs=None):
    from concourse.bass_utils import run_bass_kernel_spmd

    emb = np.asarray(emb, np.float32)
    weights = dict(
        w_a1=np.ascontiguousarray(W_a1, dtype=np.float32),
        w_a2=np.ascontiguousarray(W_a2, dtype=np.float32),
        w_c1=np.ascontiguousarray(W_c1, dtype=np.float32),
        w_c2=np.ascontiguousarray(W_c2, dtype=np.float32),
        w_g1=np.ascontiguousarray(W_g1, dtype=np.float32),
        w_g2=np.ascontiguousarray(W_g2, dtype=np.float32),
        w_o=np.ascontiguousarray(W_o, dtype=np.float32),
        b_a1=np.asarray(b_a1, np.float32).reshape(-1, 1),
        b_a2=np.asarray(b_a2, np.float32).reshape(-1, 1),
        b_c1=np.asarray(b_c1, np.float32).reshape(-1, 1),
        b_c2=np.asarray(b_c2, np.float32).reshape(-1, 1),
        b_g1=np.asarray(b_g1, np.float32).reshape(-1, 1),
        b_g2=np.asarray(b_g2, np.float32).reshape(-1, 1),
        b_o=np.asarray(b_o, np.float32).reshape(-1, 1),
    )
    in_maps = _host_prep(data, row_idx, col_idx, emb, weights)

    if "nc" not in _CACHE:
        _CACHE["nc"] = _build_graph()
    nc = _CACHE["nc"]

    kw = {}
    if _trace:
        _install_profile_shim()
        kw = dict(trace=True, trace_kwargs=_trace_kwargs or {},
                  tmpdir=_CACHE.get("trace_dir"))
    res = run_bass_kernel_spmd(nc, in_maps, list(range(N_CORES)), **kw)
    results = res.results
    P = np.concatenate([np.asarray(results[c]["out_p"]) for c in range(N_CORES)], 0)
    logits = np.concatenate(
        [np.asarray(results[c]["out_lg"]).T for c in range(N_CORES)], 0)
    _CACHE["last_exec_time_ns"] = res.exec_time_ns
    return P.astype(np.float32), logits.astype(np.float32)
